# revision 1
# baseline (speedup 1.0000x reference)
"""KNN topological BCE loss (N=8192, D=128, k=8) on 8 Trainium2 NeuronCores.

Math reformulation (validated to ~1e-6 rel against the torch/jax reference):
  loss_ij = 100*(t_ij + A_ij*(1-2 t_ij))
  mean loss = 100*(S_t + S_Au)/N^2,  S_t = sum(t),  S_Au = sum_ij A_ij*(1-2 t_ij)
where A is the symmetrized k=8 NN adjacency:
  A_ij = [d2_ij <= max(tau_i, tau_j)],  tau_i = 8th smallest off-diag d2 in row i.
On v_ij = 2*z_i.z_j - |z_j|^2  (per-row order-reversed d2; diag forced to -BIG):
  tauv_i = 8th largest of v[i,:]
  A_ij   = [v_ij >= min(tauv_i, sq_i + mtd_j)],  mtd_j = tauv_j - sq_j
so only the per-row scalars (tauv, sq, mtd) must be exchanged between cores.

Sharding: core c owns rows [c*1024, (c+1)*1024).  One matmul pass builds the
core's v block (bf16, cached in SBUF, 16MB), max8 gives row thresholds, an
AllGather shares 8192 bf16 thresholds, then a fused compare/mul/accumulate
pass streams the core's target_adj rows once.  Host sums tiny partials.
"""
import sys

sys.path.insert(0, "/opt/trn_rl_repo")

import numpy as np

import concourse.bass as bass
import concourse.mybir as mybir
import concourse.tile as tile
from concourse import bacc
from concourse.bass import ds, ts
from concourse.masks import make_identity

F32 = mybir.dt.float32
BF16 = mybir.dt.bfloat16
AF = mybir.ActivationFunctionType
OP = mybir.AluOpType

N = 8192
D = 128
NCORES = 8
R = N // NCORES          # 1024 rows per core
NSTRIP = R // 128        # 8 strips of 128 rows per core
CT = 512                 # phase-1 psum col tile
NCT = N // CT            # 16
CH = 1024                # t-stream DMA chunk width
NCH = N // CH            # 4 chunks per strip
SUB = 1024               # phase-2 DVE op width
NSUB = N // SUB          # 8 per strip
BIG = 65536.0

_CACHE = {}


def build():
    nc = bacc.Bacc("TRN2", target_bir_lowering=False, debug=False,
                   num_devices=NCORES)

    zt = nc.declare_dram_parameter("zt", [D, N], F32, isOutput=False)
    zrt = nc.declare_dram_parameter("zrt", [D, R], F32, isOutput=False)
    zr = nc.declare_dram_parameter("zr", [R, D], F32, isOutput=False)
    tm = nc.declare_dram_parameter("t", [R, N], F32, isOutput=False)
    sau_out = nc.declare_dram_parameter("sau", [128, NSTRIP * NSUB], F32,
                                        isOutput=True)
    su_out = nc.declare_dram_parameter("su", [128, NSTRIP * NCH], F32,
                                       isOutput=True)

    cc_in = nc.dram_tensor("cc_in", [R], BF16)
    cc_out = nc.dram_tensor("cc_out", [N], BF16, addr_space="Shared")

    with tile.TileContext(nc) as tc:
        with tc.tile_pool(name="const", bufs=1) as const, \
             tc.tile_pool(name="vpool", bufs=1) as vpool, \
             tc.tile_pool(name="stream", bufs=2) as stream, \
             tc.tile_pool(name="work", bufs=2) as work, \
             tc.tile_pool(name="psum", bufs=4, space="PSUM") as psum, \
             tc.tile_pool(name="psmall", bufs=2, space="PSUM") as psmall:

            # ---------- constants ----------
            ones1 = const.tile([1, 128], BF16)
            nc.gpsimd.memset(ones1[:, :], 1.0)
            ones_col = const.tile([128, 1], BF16)
            nc.gpsimd.memset(ones_col[:, :], 1.0)
            ident = const.tile([128, 128], BF16)
            make_identity(nc, ident[:, :])
            mbig = const.tile([128, 128], BF16)
            nc.vector.tensor_scalar_mul(mbig[:, :], ident[:, :], -BIG)

            # ---------- setup: ZT bf16, lhsT2, -sq_j row ----------
            ztb = const.tile([128, N], BF16, tag="big8k")
            for i in range(N // SUB):
                ztf = stream.tile([128, SUB], F32, tag="ld")
                nc.sync.dma_start(out=ztf[:, :], in_=zt[:, ts(i, SUB)])
                nc.vector.tensor_copy(ztb[:, ts(i, SUB)], ztf[:, :])

            lhsT2 = const.tile([128, R], BF16)
            zrtf = stream.tile([128, R], F32, tag="zrt")
            nc.sync.dma_start(out=zrtf[:, :], in_=zrt[:, :])
            nc.vector.tensor_scalar_mul(lhsT2[:, :], zrtf[:, :], 2.0)

            msq_row = const.tile([1, N], BF16, tag="row8k")
            for c in range(NCT):
                zsq = work.tile([128, CT], BF16, tag="zsq")
                nc.scalar.activation(zsq[:, :], ztb[:, ts(c, CT)], AF.Square)
                ps_sq = psmall.tile([1, CT], F32, tag="pssq")
                nc.tensor.matmul(ps_sq[:, :], ones_col[:, :], zsq[:, :],
                                 start=True, stop=True)
                nc.scalar.activation(msq_row[:, ts(c, CT)], ps_sq[:, :],
                                     AF.Copy, scale=-1.0)

            # per-strip v tiles (8 x 16KB/partition = 128KB/partition)
            vch = [vpool.tile([128, N], BF16, tag=f"v{s}", name=f"vch{s}")
                   for s in range(NSTRIP)]

            tauv = const.tile([128, NSTRIP], F32)
            sqp = const.tile([128, NSTRIP], F32)
            sau_cols = const.tile([128, NSTRIP * NSUB], F32)
            su_cols = const.tile([128, NSTRIP * NCH], F32)

            pid = nc.vector.partition_id()
            rowbase = pid * R

            # ---------- phase 1: v blocks + row thresholds ----------
            for s in range(NSTRIP):
                zrf = stream.tile([128, D], F32, tag="zr")
                nc.sync.dma_start(out=zrf[:, :], in_=zr[ts(s, 128), :])
                zsq2 = work.tile([128, D], F32, tag="zsq2")
                nc.scalar.activation(zsq2[:, :], zrf[:, :], AF.Square,
                                     accum_out=sqp[:, s:s + 1])

                for c in range(NCT):
                    ps = psum.tile([128, CT], F32, tag="ps")
                    nc.tensor.matmul(ps[:, :], lhsT2[:, ts(s, 128)],
                                     ztb[:, ts(c, CT)], start=True, stop=False)
                    nc.tensor.matmul(ps[:, :], ones1[:, :],
                                     msq_row[:, ts(c, CT)],
                                     start=False, stop=True)
                    nc.scalar.activation(vch[s][:, ts(c, CT)], ps[:, :],
                                         AF.Copy)

                # diagonal -> -BIG: in-place add of -BIG*I at dynamic offset
                dcol = rowbase + (s * 128)
                nc.vector.tensor_tensor(
                    vch[s][:, ds(dcol, 128)], vch[s][:, ds(dcol, 128)],
                    mbig[:, :], OP.add)

                v8 = work.tile([128, 8], BF16, tag="v8")
                nc.vector.max(v8[:, :], vch[s][:, :])
                nc.vector.tensor_copy(tauv[:, s:s + 1], v8[:, 7:8])
                mtd = work.tile([128, 1], F32, tag="mtd")
                nc.vector.tensor_tensor(mtd[:, :], tauv[:, s:s + 1],
                                        sqp[:, s:s + 1], OP.subtract)
                mtdb_s = work.tile([128, 1], BF16, tag="mtdb")
                nc.vector.tensor_copy(mtdb_s[:, :], mtd[:, :])
                nc.sync.dma_start(out=cc_in[ts(s, 128)], in_=mtdb_s[:, :])

            # ---------- all-gather thresholds (mtd_j = tauv_j - sq_j) ------
            nc.gpsimd.collective_compute(
                "AllGather", OP.bypass,
                replica_groups=[list(range(NCORES))],
                ins=[cc_in[:].opt()],
                outs=[cc_out[:].opt()],
            )
            mtd_row = const.tile([1, N], BF16, tag="row8k")
            nc.sync.dma_start(out=mtd_row[:, :], in_=cc_out[:])

            mtdb = const.tile([128, N], BF16, tag="big8k")
            for c in range(NCT):
                psb = psum.tile([128, CT], F32, tag="ps")
                nc.tensor.matmul(psb[:, :], ones1[:, :],
                                 mtd_row[:, ts(c, CT)], start=True, stop=True)
                nc.scalar.activation(mtdb[:, ts(c, CT)], psb[:, :], AF.Copy)

            # ---------- phase 2: fused masked accumulation ----------
            for s in range(NSTRIP):
                for ch in range(NCH):
                    tt = stream.tile([128, CH], F32, tag="ld")
                    nc.sync.dma_start(out=tt[:, :],
                                      in_=tm[ts(s, 128), ts(ch, CH)])
                    ut = work.tile([128, CH], BF16, tag="u")
                    nc.scalar.activation(
                        ut[:, :], tt[:, :], AF.Copy, scale=-2.0, bias=1.0,
                        accum_out=su_cols[:, s * NCH + ch: s * NCH + ch + 1])
                    for k in range(CH // SUB):
                        j0 = ch * CH + k * SUB
                        ci = s * NSUB + j0 // SUB
                        thr2 = work.tile([128, SUB], BF16, tag="thr2")
                        nc.vector.tensor_scalar(
                            thr2[:, :], mtdb[:, j0:j0 + SUB],
                            sqp[:, s:s + 1], tauv[:, s:s + 1],
                            OP.add, OP.min)
                        At = work.tile([128, SUB], BF16, tag="A")
                        nc.vector.tensor_tensor(
                            At[:, :], vch[s][:, j0:j0 + SUB], thr2[:, :],
                            OP.is_ge)
                        nc.vector.scalar_tensor_tensor(
                            thr2[:, :], At[:, :], 1.0,
                            ut[:, k * SUB:(k + 1) * SUB],
                            OP.mult, OP.mult,
                            accum_out=sau_cols[:, ci:ci + 1])

            nc.sync.dma_start(out=sau_out[:, :], in_=sau_cols[:, :])
            nc.sync.dma_start(out=su_out[:, :], in_=su_cols[:, :])

    nc.finalize()
    return nc


def _make_exec(nc):
    """Cached jitted SPMD executor (mirrors bass2jax.run_bass_via_pjrt)."""
    import jax
    from jax.sharding import Mesh, PartitionSpec
    try:
        from jax.experimental.shard_map import shard_map
    except Exception:
        from jax.sharding import shard_map  # newer jax
    from concourse import bass2jax

    bass2jax.install_neuronx_cc_hook()

    partition_name = (nc.partition_id_tensor.name
                      if nc.partition_id_tensor else None)
    in_names, out_names, out_avals, zero_out_shapes = [], [], [], []
    for alloc in nc.m.functions[0].allocations:
        if not isinstance(alloc, mybir.MemoryLocationSet):
            continue
        name = alloc.memorylocations[0].name
        if alloc.kind == "ExternalInput":
            if name != partition_name:
                in_names.append(name)
        elif alloc.kind == "ExternalOutput":
            shape = tuple(alloc.tensor_shape)
            dtype = mybir.dt.np(alloc.dtype)
            out_names.append(name)
            out_avals.append(jax.core.ShapedArray(shape, dtype))
            zero_out_shapes.append((shape, dtype))
    n_params = len(in_names)
    n_outs = len(out_names)
    all_in_names = list(in_names) + list(out_names)
    if partition_name is not None:
        all_in_names.append(partition_name)
    donate = tuple(range(n_params, n_params + n_outs))

    def _body(*args):
        operands = list(args)
        if partition_name is not None:
            operands.append(bass2jax.partition_id_tensor())
        outs = bass2jax._bass_exec_p.bind(
            *operands,
            out_avals=tuple(out_avals),
            in_names=tuple(all_in_names),
            out_names=tuple(out_names),
            lowering_input_output_aliases=(),
            sim_require_finite=True,
            sim_require_nnan=True,
            nc=nc,
        )
        return tuple(outs)

    devices = jax.devices()[:NCORES]
    mesh = Mesh(np.asarray(devices), ("core",))
    in_specs = (PartitionSpec("core"),) * (n_params + n_outs)
    out_specs = (PartitionSpec("core"),) * n_outs
    sharded = jax.jit(
        shard_map(_body, mesh=mesh, in_specs=in_specs, out_specs=out_specs,
                  check_rep=False),
        donate_argnums=donate, keep_unused=True)

    _CACHE["sharded"] = sharded

    def runner(in_maps):
        concat_in = [np.concatenate([np.asarray(m[nm]) for m in in_maps],
                                    axis=0) for nm in in_names]
        zeros = [np.zeros((NCORES * sh[0],) + tuple(sh[1:]), dt)
                 for sh, dt in zero_out_shapes]
        out_arrs = sharded(*concat_in, *zeros)
        res = []
        for c in range(NCORES):
            d = {}
            for i, nm in enumerate(out_names):
                a = np.asarray(out_arrs[i])
                per = a.shape[0] // NCORES
                d[nm] = a[c * per:(c + 1) * per]
            res.append(d)
        return res

    return runner


def _get_runner():
    if "runner" not in _CACHE:
        nc = build()
        _CACHE["runner"] = _make_exec(nc)
    return _CACHE["runner"]


def _prep_inputs(Z, T):
    Z = np.ascontiguousarray(np.asarray(Z, dtype=np.float32))
    T = np.asarray(target_adj_as_f32(T))
    ZT = np.ascontiguousarray(Z.T)  # [D, N]
    in_maps = []
    for c in range(NCORES):
        in_maps.append({
            "zt": ZT,
            "zrt": np.ascontiguousarray(ZT[:, c * R:(c + 1) * R]),
            "zr": Z[c * R:(c + 1) * R],
            "t": T[c * R:(c + 1) * R],
        })
    return in_maps


def target_adj_as_f32(T):
    T = np.asarray(T)
    if T.dtype != np.float32:
        T = T.astype(np.float32)
    return T


def assemble_loss(results):
    s_au = 0.0
    s_u = 0.0
    for r in results:
        s_au += float(np.asarray(r["sau"], dtype=np.float64).sum())
        s_u += float(np.asarray(r["su"], dtype=np.float64).sum())
    s_t = (float(N) * N - s_u) / 2.0
    return np.float32(100.0 * (s_t + s_au) / (float(N) * N))


def kernel(Z, target_adj):
    runner = _get_runner()
    in_maps = _prep_inputs(Z, target_adj)
    results = runner(in_maps)
    return assemble_loss(results)


if __name__ == "__main__":
    rng = np.random.default_rng(0)
    Z = rng.standard_normal((N, D), dtype=np.float32)
    T = rng.random((N, N), dtype=np.float32)
    print("loss:", kernel(Z, T))



# revision 39
# speedup vs baseline: 34.9481x; 34.9481x over previous
"""KNN topological BCE loss (N=8192, D=128, k=8) on 8 Trainium2 NeuronCores.

Math reformulation (validated to ~1e-6 rel against the torch/jax reference):
  loss_ij = 100*(t_ij + A_ij*(1-2 t_ij))
  mean loss = 100*(S_t + S_Au)/N^2,  S_t = sum(t),  S_Au = sum_ij A_ij*u_ij,
  u = 1-2t
where A is the symmetrized k=8 NN adjacency:
  A_ij = [d2_ij <= max(tau_i, tau_j)],  tau_i = 8th smallest off-diag d2 row i.
On v_ij = 2*z_i.z_j - |z_j|^2  (per-row order-reversed d2; diag forced -BIG):
  tauv_i = 8th largest of v[i,:]
  A_ij   = [v_ij >= min(tauv_i, sq_i + mtd_j)],  mtd_j = tauv_j - sq_j
so only per-row scalars (tauv, sq, mtd) are exchanged between cores.

Per-core schedule (rows [c*1024,(c+1)*1024)):
  P1: PE matmuls build v (bf16, 16MB SBUF) + ACT psum->SBUF copies + DVE
      max8 row thresholds; host pre-computes bf16 Z^T, -|z_j|^2 row, |z_i|^2.
  AllGather of 8192 bf16 thresholds (mtd).
  P2: stream target_adj once as w = t-0.5 (DVE single-op tensor_scalar_sub
      / ACT bias; the DVE two-op tensor_scalar drops op1 on real HW and
      Pool rejects TensorTensor, both found the hard way),
      thr=min(mtd_j+sq_i,tauv_i) (DVE TSP 4x), A=[v>=thr] (DVE TT 2x),
      then both sums ride the TENSOR engine: psum += A_blk^T @ w_blk per
      128-col block puts sum(A.w) on the diagonal of one [128,128]
      accumulator (S_Au = -2 tr), and ones^T @ w col-sums accumulate S_w
      (S_t = S_w + N^2/2).  Host sums the tiny outputs.
"""
import sys

sys.path.insert(0, "/opt/trn_rl_repo")

import numpy as np
import ml_dtypes

import concourse.bass as bass
import concourse.mybir as mybir
import concourse.tile as tile
from concourse import bacc
from concourse.bass import ds, ts
from concourse.masks import make_identity

F32 = mybir.dt.float32
BF16 = mybir.dt.bfloat16
AF = mybir.ActivationFunctionType
OP = mybir.AluOpType

N = 8192
D = 128
NCORES = 8
R = N // NCORES          # 1024 rows per core
NSTRIP = R // 128        # 8 strips of 128 rows
CT = 512                 # matmul col tile (one psum bank)
PG = 1024                # psum group width (2 banks) per ACT copy
NPG = N // PG            # 8 groups per strip
CH = 2048                # phase-2 chunk width
NCH = N // CH            # 4 chunks per strip
NIT = NSTRIP * NCH       # 32 phase-2 iterations
NB = CH // 128           # 16 diag-matmul blocks per iteration
BIG = 65536.0

PF = 1                   # iterations prefetched (DMA+uconv) before phase 1
POOL_ISGE_MOD = 4        # is_ge on Pool unless it % MOD == 0 (24/32 on pool)

_CACHE = {}


def build(sim_nocc=False, debug_taps=False):
    nc = bacc.Bacc("TRN2", target_bir_lowering=False, debug=False,
                   num_devices=NCORES)
    dbg = {}
    if debug_taps:
        dbg["thr"] = nc.declare_dram_parameter("dthr", [128, CH], BF16,
                                               isOutput=True)
        dbg["A"] = nc.declare_dram_parameter("dA", [128, CH], BF16,
                                             isOutput=True)
        dbg["u"] = nc.declare_dram_parameter("du", [128, CH], BF16,
                                             isOutput=True)
        dbg["mtdb"] = nc.declare_dram_parameter("dmtdb", [128, CH], BF16,
                                                isOutput=True)
        dbg["v"] = nc.declare_dram_parameter("dv", [128, CH], BF16,
                                             isOutput=True)
        dbg["tauv"] = nc.declare_dram_parameter("dtauv", [128, NSTRIP], F32,
                                                isOutput=True)

    ztb_in = nc.declare_dram_parameter("ztb", [128, N], BF16, isOutput=False)
    l2t_in = nc.declare_dram_parameter("l2t", [128, R], BF16, isOutput=False)
    msq_in = nc.declare_dram_parameter("msq", [1, N], BF16, isOutput=False)
    sq_in = nc.declare_dram_parameter("sq", [128, NSTRIP], F32, isOutput=False)
    tm = nc.declare_dram_parameter("t", [R, N], F32, isOutput=False)
    sau_out = nc.declare_dram_parameter("sau", [128, 128], F32, isOutput=True)
    st_out = nc.declare_dram_parameter("st", [1, 512], F32, isOutput=True)

    cc_in = nc.dram_tensor("cc_in", [R], BF16)
    cc_out = nc.dram_tensor("cc_out", [N], BF16, addr_space="Shared")

    with tile.TileContext(nc) as tc:
        with tc.tile_pool(name="const", bufs=1) as const, \
             tc.tile_pool(name="vpool", bufs=1) as vpool, \
             tc.tile_pool(name="tstream", bufs=3) as tstream, \
             tc.tile_pool(name="upool", bufs=PF + 1) as upool, \
             tc.tile_pool(name="apool", bufs=2) as apool, \
             tc.tile_pool(name="work", bufs=2) as work, \
             tc.tile_pool(name="vps", bufs=2, space="PSUM") as vps, \
             tc.tile_pool(name="dps", bufs=1, space="PSUM") as dps:

            # ---------- constants / persistent ----------
            ones1 = const.tile([1, 128], BF16)
            nc.gpsimd.memset(ones1[:, :], 1.0)
            mbig1 = const.tile([128, 1], F32)
            nc.gpsimd.memset(mbig1[:, :], -BIG)

            ztb = const.tile([128, N], BF16, tag="big8k")
            l2t = const.tile([128, R], BF16)
            nc.scalar.dma_start(out=l2t[:, :], in_=l2t_in[:, :])
            msq_row = const.tile([1, N], BF16, tag="row8k")
            nc.scalar.dma_start(out=msq_row[:, :], in_=msq_in[:, :])
            sqp = const.tile([128, NSTRIP], F32)
            nc.scalar.dma_start(out=sqp[:, :], in_=sq_in[:, :])
            smargin = const.tile([128, NSTRIP], F32)
            nc.vector.tensor_scalar_sub(smargin[:, :], sqp[:, :], 1.0)
            # split ztb load so the first matmuls start early
            for zc in range(4):
                nc.sync.dma_start(out=ztb[:, ts(zc, N // 4)],
                                  in_=ztb_in[:, ts(zc, N // 4)])

            vch = [vpool.tile([128, N], BF16, tag=f"v{s}", name=f"vch{s}")
                   for s in range(NSTRIP)]
            tauv = const.tile([128, NSTRIP], F32)
            ones_col = const.tile([128, 1], BF16)
            nc.gpsimd.memset(ones_col[:, :], 1.0)

            sau_sb = const.tile([128, 128], F32)
            st_sb = const.tile([1, 512], F32)

            pid = nc.vector.partition_id()
            rowbase = pid * R

            # t-loads round-robin across issuing engines -> separate HWDGE
            # queues, so transfers overlap instead of serializing at depth 1
            dma_eng = [nc.sync, nc.scalar]

            # ---------- prefetch: first PF iterations' t-load + uconv ------
            uts = {}
            for it in range(PF):
                s, c = divmod(it, NCH)
                tt = tstream.tile([128, CH], F32, tag="t")
                dma_eng[it % 2].dma_start(out=tt[:, :],
                                          in_=tm[ts(s, 128), ts(c, CH)])
                ut = upool.tile([128, CH], BF16, tag="u")
                nc.vector.tensor_scalar_sub(ut[:, :], tt[:, :], 0.5)
                uts[it] = ut

            # ---------- phase 1: v blocks + row thresholds ----------
            # per-group top-8 candidates pipeline with the psum copies; the
            # self column v_ii = |z_i|^2 is the strict row max (d2>0), so it
            # is masked to -BIG in the tiny candidate tile instead of vch
            # (A_ii=1 in phase 2 is corrected exactly on the host).
            for s in range(NSTRIP):
                v8g = work.tile([128, 8 * NPG], BF16, tag="v8g")
                for g in range(NPG):
                    ps = vps.tile([128, PG], F32, tag="vps")
                    for h in range(PG // CT):
                        c0 = g * PG + h * CT
                        nc.tensor.matmul(ps[:, ts(h, CT)], l2t[:, ts(s, 128)],
                                         ztb[:, ds(c0, CT)],
                                         start=True, stop=False)
                        nc.tensor.matmul(ps[:, ts(h, CT)], ones1[:, :],
                                         msq_row[:, ds(c0, CT)],
                                         start=False, stop=True)
                    nc.scalar.activation(vch[s][:, ts(g, PG)], ps[:, :],
                                         AF.Copy)
                    nc.vector.max(v8g[:, ts(g, 8)], vch[s][:, ts(g, PG)])

                pen = work.tile([128, 8 * NPG], BF16, tag="pen")
                nc.vector.tensor_scalar(pen[:, :], v8g[:, :],
                                        smargin[:, s:s + 1], mbig1[:, :],
                                        OP.is_ge, OP.mult)
                nc.vector.tensor_tensor(v8g[:, :], v8g[:, :], pen[:, :],
                                        OP.add)
                v8 = work.tile([128, 8], BF16, tag="v8")
                nc.vector.max(v8[:, :], v8g[:, :])
                nc.vector.tensor_copy(tauv[:, s:s + 1], v8[:, 7:8])
                mtd = work.tile([128, 1], F32, tag="mtd")
                nc.vector.tensor_tensor(mtd[:, :], tauv[:, s:s + 1],
                                        sqp[:, s:s + 1], OP.subtract)
                mtdb_s = work.tile([128, 1], BF16, tag="mtdb1")
                nc.vector.tensor_copy(mtdb_s[:, :], mtd[:, :])
                nc.sync.dma_start(out=cc_in[ts(s, 128)], in_=mtdb_s[:, :])

            # load-only prefetch: next 2 t-chunks issued before the
            # collective so the stream is not serialized behind it
            tts = {}
            for it in range(PF, PF + 2):
                s, c = divmod(it, NCH)
                tt = tstream.tile([128, CH], F32, tag="t")
                dma_eng[it % 2].dma_start(out=tt[:, :],
                                          in_=tm[ts(s, 128), ts(c, CH)])
                tts[it] = tt

            # ---------- all-gather thresholds (mtd_j = tauv_j - sq_j) ------
            if sim_nocc:
                for c in range(NCORES):
                    nc.sync.dma_start(out=cc_out[ts(c, R)], in_=cc_in[:])
            else:
                nc.gpsimd.collective_compute(
                    "AllGather", OP.bypass,
                    replica_groups=[list(range(NCORES))],
                    ins=[cc_in[:].opt()],
                    outs=[cc_out[:].opt()],
                )
            mtd_row = const.tile([1, N], BF16, tag="row8k")
            nc.sync.dma_start(out=mtd_row[:, :], in_=cc_out[:])

            mtdb = const.tile([128, N], BF16, tag="big8k")
            for g in range(4):
                nc.gpsimd.partition_broadcast(mtdb[:, ts(g, N // 4)],
                                              mtd_row[:, ts(g, N // 4)])

            # ---------- phase 2: masked accumulation ----------
            dpsum = dps.tile([128, 128], F32, name="dpsum")
            stps = dps.tile([1, 512], F32, name="stps")
            for it in range(NIT):
                s, c = divmod(it, NCH)
                if it in uts:
                    ut = uts[it]
                else:
                    if it in tts:
                        tt = tts[it]
                    else:
                        tt = tstream.tile([128, CH], F32, tag="t")
                        dma_eng[it % 2].dma_start(
                            out=tt[:, :], in_=tm[ts(s, 128), ts(c, CH)])
                    ut = upool.tile([128, CH], BF16, tag="u")
                    if it % 2 == 1:
                        nc.scalar.activation(ut[:, :], tt[:, :], AF.Copy,
                                             bias=-0.5)
                    else:
                        nc.vector.tensor_scalar_sub(ut[:, :], tt[:, :], 0.5)

                At = apool.tile([128, CH], BF16, tag="A")
                nc.vector.tensor_scalar(At[:, :], mtdb[:, ts(c, CH)],
                                        sqp[:, s:s + 1], tauv[:, s:s + 1],
                                        OP.add, OP.min)
                if debug_taps and it == 0:
                    nc.sync.dma_start(out=dbg["thr"][:, :], in_=At[:, :])
                    nc.sync.dma_start(out=dbg["u"][:, :], in_=ut[:, :])
                    nc.sync.dma_start(out=dbg["mtdb"][:, :],
                                      in_=mtdb[:, ts(c, CH)])
                    nc.sync.dma_start(out=dbg["v"][:, :],
                                      in_=vch[s][:, ts(c, CH)])
                    nc.sync.dma_start(out=dbg["tauv"][:, :], in_=tauv[:, :])
                nc.vector.tensor_tensor(At[:, :], vch[s][:, ts(c, CH)],
                                        At[:, :], OP.is_ge)
                if debug_taps and it == 0:
                    nc.sync.dma_start(out=dbg["A"][:, :], in_=At[:, :])

                for b in range(NB):
                    nc.tensor.matmul(dpsum[:, :], At[:, ts(b, 128)],
                                     ut[:, ts(b, 128)],
                                     start=(it == 0 and b == 0),
                                     stop=(it == NIT - 1 and b == NB - 1))
                for h in range(CH // 512):
                    nc.tensor.matmul(stps[:, :], ones_col[:, :],
                                     ut[:, ts(h, 512)],
                                     start=(it == 0 and h == 0),
                                     stop=(it == NIT - 1 and h == CH // 512 - 1))

            nc.scalar.activation(sau_sb[:, :], dpsum[:, :], AF.Copy)
            nc.scalar.activation(st_sb[:, :], stps[:, :], AF.Copy)
            nc.sync.dma_start(out=sau_out[:, :], in_=sau_sb[:, :])
            nc.sync.dma_start(out=st_out[:, :], in_=st_sb[:, :])

    nc.finalize()
    return nc


def _make_exec(nc):
    """Cached jitted SPMD executor (mirrors bass2jax.run_bass_via_pjrt)."""
    import jax
    from jax.sharding import Mesh, PartitionSpec
    try:
        from jax.experimental.shard_map import shard_map
    except Exception:
        from jax.sharding import shard_map  # newer jax
    from concourse import bass2jax

    bass2jax.install_neuronx_cc_hook()

    partition_name = (nc.partition_id_tensor.name
                      if nc.partition_id_tensor else None)
    in_names, out_names, out_avals, zero_out_shapes = [], [], [], []
    for alloc in nc.m.functions[0].allocations:
        if not isinstance(alloc, mybir.MemoryLocationSet):
            continue
        name = alloc.memorylocations[0].name
        if alloc.kind == "ExternalInput":
            if name != partition_name:
                in_names.append(name)
        elif alloc.kind == "ExternalOutput":
            shape = tuple(alloc.tensor_shape)
            dtype = mybir.dt.np(alloc.dtype)
            out_names.append(name)
            out_avals.append(jax.core.ShapedArray(shape, dtype))
            zero_out_shapes.append((shape, dtype))
    n_params = len(in_names)
    n_outs = len(out_names)
    all_in_names = list(in_names) + list(out_names)
    if partition_name is not None:
        all_in_names.append(partition_name)
    donate = tuple(range(n_params, n_params + n_outs))

    def _body(*args):
        operands = list(args)
        if partition_name is not None:
            operands.append(bass2jax.partition_id_tensor())
        outs = bass2jax._bass_exec_p.bind(
            *operands,
            out_avals=tuple(out_avals),
            in_names=tuple(all_in_names),
            out_names=tuple(out_names),
            lowering_input_output_aliases=(),
            sim_require_finite=True,
            sim_require_nnan=True,
            nc=nc,
        )
        return tuple(outs)

    devices = jax.devices()[:NCORES]
    mesh = Mesh(np.asarray(devices), ("core",))
    in_specs = (PartitionSpec("core"),) * (n_params + n_outs)
    out_specs = (PartitionSpec("core"),) * n_outs
    sharded = jax.jit(
        shard_map(_body, mesh=mesh, in_specs=in_specs, out_specs=out_specs,
                  check_rep=False),
        donate_argnums=donate, keep_unused=True)

    _CACHE["sharded"] = sharded
    _CACHE["in_names"] = in_names
    _CACHE["zero_out_shapes"] = zero_out_shapes
    _CACHE["out_names"] = out_names

    def runner(in_maps):
        concat_in = [np.concatenate([np.asarray(m[nm]) for m in in_maps],
                                    axis=0) for nm in in_names]
        zeros = [np.zeros((NCORES * sh[0],) + tuple(sh[1:]), dt)
                 for sh, dt in zero_out_shapes]
        out_arrs = sharded(*concat_in, *zeros)
        res = []
        for c in range(NCORES):
            d = {}
            for i, nm in enumerate(out_names):
                a = np.asarray(out_arrs[i])
                per = a.shape[0] // NCORES
                d[nm] = a[c * per:(c + 1) * per]
            res.append(d)
        return res

    return runner


def _get_runner():
    if "runner" not in _CACHE:
        nc = build()
        _CACHE["runner"] = _make_exec(nc)
    return _CACHE["runner"]


def _prep_inputs(Z, T):
    Z = np.ascontiguousarray(np.asarray(Z, dtype=np.float32))
    T = np.asarray(T)
    if T.dtype != np.float32:
        T = T.astype(np.float32)
    bf16 = ml_dtypes.bfloat16
    _CACHE["diag_corr"] = float(
        np.sum(np.diagonal(T).astype(np.float64) - 0.5))
    ZT = np.ascontiguousarray(Z.T)                       # [D, N] f32
    ztb = ZT.astype(bf16)                                # [128, N] bf16
    sq = np.sum(Z.astype(np.float64) * Z, axis=1).astype(np.float32)  # [N]
    msq = (-sq).astype(bf16)[None, :]                    # [1, N] bf16
    in_maps = []
    for c in range(NCORES):
        rows = slice(c * R, (c + 1) * R)
        l2t = np.ascontiguousarray((2.0 * ZT[:, rows]).astype(bf16))
        sqc = np.ascontiguousarray(
            sq[rows].reshape(NSTRIP, 128).T)             # [128, NSTRIP]
        in_maps.append({
            "ztb": ztb,
            "l2t": l2t,
            "msq": msq,
            "sq": sqc,
            "t": T[rows],
        })
    return in_maps


def assemble_loss(results):
    s_aw = 0.0
    s_w = 0.0
    for r in results:
        s_aw += float(np.asarray(r["sau"], dtype=np.float64)
                      .diagonal().sum())
        s_w += float(np.asarray(r["st"], dtype=np.float64).sum())
    # A_ii = 1 on device (self not masked in vch); remove its contribution
    s_aw -= _CACHE.get("diag_corr", 0.0)
    s_au = -2.0 * s_aw
    s_t = s_w + 0.5 * float(N) * N
    return np.float32(100.0 * (s_t + s_au) / (float(N) * N))


def kernel(Z, target_adj):
    runner = _get_runner()
    in_maps = _prep_inputs(Z, target_adj)
    results = runner(in_maps)
    return assemble_loss(results)


if __name__ == "__main__":
    rng = np.random.default_rng(0)
    Z = rng.standard_normal((N, D), dtype=np.float32)
    T = rng.random((N, N), dtype=np.float32)
    print("loss:", kernel(Z, T))


# revision 40
# speedup vs baseline: 39.2524x; 1.1232x over previous
"""KNN topological BCE loss (N=8192, D=128, k=8) on 8 Trainium2 NeuronCores.

Math reformulation (validated to ~1e-6 rel against the torch/jax reference):
  loss_ij = 100*(t_ij + A_ij*(1-2 t_ij))
  mean loss = 100*(S_t + S_Au)/N^2,  S_t = sum(t),  S_Au = sum_ij A_ij*u_ij,
  u = 1-2t
where A is the symmetrized k=8 NN adjacency:
  A_ij = [d2_ij <= max(tau_i, tau_j)],  tau_i = 8th smallest off-diag d2 row i.
On v_ij = 2*z_i.z_j - |z_j|^2  (per-row order-reversed d2; diag forced -BIG):
  tauv_i = 8th largest of v[i,:]
  A_ij   = [v_ij >= min(tauv_i, sq_i + mtd_j)],  mtd_j = tauv_j - sq_j
so only per-row scalars (tauv, sq, mtd) are exchanged between cores.

Per-core schedule (rows [c*1024,(c+1)*1024)):
  P1: PE matmuls build v (bf16, 16MB SBUF) + ACT psum->SBUF copies + DVE
      max8 row thresholds; host pre-computes bf16 Z^T, -|z_j|^2 row, |z_i|^2.
  AllGather of 8192 bf16 thresholds (mtd).
  P2: stream target_adj once as w = t-0.5 (DVE single-op tensor_scalar_sub
      / ACT bias; the DVE two-op tensor_scalar drops op1 on real HW and
      Pool rejects TensorTensor, both found the hard way),
      thr=min(mtd_j+sq_i,tauv_i) (DVE TSP 4x), A=[v>=thr] (DVE TT 2x),
      then both sums ride the TENSOR engine: psum += A_blk^T @ w_blk per
      128-col block puts sum(A.w) on the diagonal of one [128,128]
      accumulator (S_Au = -2 tr), and ones^T @ w col-sums accumulate S_w
      (S_t = S_w + N^2/2).  Host sums the tiny outputs.
"""
import sys

sys.path.insert(0, "/opt/trn_rl_repo")

import numpy as np
import ml_dtypes

import concourse.bass as bass
import concourse.mybir as mybir
import concourse.tile as tile
from concourse import bacc
from concourse.bass import ds, ts
from concourse.masks import make_identity

F32 = mybir.dt.float32
BF16 = mybir.dt.bfloat16
AF = mybir.ActivationFunctionType
OP = mybir.AluOpType

N = 8192
D = 128
NCORES = 8
R = N // NCORES          # 1024 rows per core
NSTRIP = R // 128        # 8 strips of 128 rows
CT = 512                 # matmul col tile (one psum bank)
PG = 1024                # psum group width (2 banks) per ACT copy
NPG = N // PG            # 8 groups per strip
CH = 2048                # phase-2 chunk width
NCH = N // CH            # 4 chunks per strip
NIT = NSTRIP * NCH       # 32 phase-2 iterations
NB = CH // 128           # 16 diag-matmul blocks per iteration
BIG = 65536.0

PF = 1                   # iterations prefetched (DMA+uconv) before phase 1
POOL_ISGE_MOD = 4        # is_ge on Pool unless it % MOD == 0 (24/32 on pool)

_CACHE = {}


def build(sim_nocc=False, debug_taps=False):
    nc = bacc.Bacc("TRN2", target_bir_lowering=False, debug=False,
                   num_devices=NCORES)
    dbg = {}
    if debug_taps:
        dbg["thr"] = nc.declare_dram_parameter("dthr", [128, CH], BF16,
                                               isOutput=True)
        dbg["A"] = nc.declare_dram_parameter("dA", [128, CH], BF16,
                                             isOutput=True)
        dbg["u"] = nc.declare_dram_parameter("du", [128, CH], BF16,
                                             isOutput=True)
        dbg["mtdb"] = nc.declare_dram_parameter("dmtdb", [128, CH], BF16,
                                                isOutput=True)
        dbg["v"] = nc.declare_dram_parameter("dv", [128, CH], BF16,
                                             isOutput=True)
        dbg["tauv"] = nc.declare_dram_parameter("dtauv", [128, NSTRIP], F32,
                                                isOutput=True)

    ztb_in = nc.declare_dram_parameter("ztb", [128, N], BF16, isOutput=False)
    l2t_in = nc.declare_dram_parameter("l2t", [128, R], BF16, isOutput=False)
    msq_in = nc.declare_dram_parameter("msq", [1, N], BF16, isOutput=False)
    sq_in = nc.declare_dram_parameter("sq", [128, NSTRIP], F32, isOutput=False)
    tm = nc.declare_dram_parameter("t", [R, N], F32, isOutput=False)
    sau_out = nc.declare_dram_parameter("sau", [128, 128], F32, isOutput=True)
    st_out = nc.declare_dram_parameter("st", [1, 512], F32, isOutput=True)

    cc_in = nc.dram_tensor("cc_in", [R], BF16)
    cc_out = nc.dram_tensor("cc_out", [N], BF16, addr_space="Shared")

    with tile.TileContext(nc) as tc:
        with tc.tile_pool(name="const", bufs=1) as const, \
             tc.tile_pool(name="vpool", bufs=1) as vpool, \
             tc.tile_pool(name="tstream", bufs=3) as tstream, \
             tc.tile_pool(name="upool", bufs=PF + 1) as upool, \
             tc.tile_pool(name="apool", bufs=2) as apool, \
             tc.tile_pool(name="work", bufs=2) as work, \
             tc.tile_pool(name="vps", bufs=2, space="PSUM") as vps, \
             tc.tile_pool(name="dps", bufs=1, space="PSUM") as dps:

            # ---------- constants / persistent ----------
            ones1 = const.tile([1, 128], BF16)
            nc.gpsimd.memset(ones1[:, :], 1.0)
            mbig1 = const.tile([128, 1], F32)
            nc.gpsimd.memset(mbig1[:, :], -BIG)

            ztb = const.tile([128, N], BF16, tag="big8k")
            l2t = const.tile([128, R], BF16)
            nc.scalar.dma_start(out=l2t[:, :], in_=l2t_in[:, :])
            msq_row = const.tile([1, N], BF16, tag="row8k")
            nc.scalar.dma_start(out=msq_row[:, :], in_=msq_in[:, :])
            sqp = const.tile([128, NSTRIP], F32)
            nc.scalar.dma_start(out=sqp[:, :], in_=sq_in[:, :])
            smargin = const.tile([128, NSTRIP], F32)
            nc.vector.tensor_scalar_sub(smargin[:, :], sqp[:, :], 1.0)
            # split ztb load so the first matmuls start early
            for zc in range(4):
                nc.sync.dma_start(out=ztb[:, ts(zc, N // 4)],
                                  in_=ztb_in[:, ts(zc, N // 4)])

            vch = [vpool.tile([128, N], BF16, tag=f"v{s}", name=f"vch{s}")
                   for s in range(NSTRIP)]
            tauv = const.tile([128, NSTRIP], F32)
            ones_col = const.tile([128, 1], BF16)
            nc.gpsimd.memset(ones_col[:, :], 1.0)

            sau_sb = const.tile([128, 128], F32)
            st_sb = const.tile([1, 512], F32)

            pid = nc.vector.partition_id()
            rowbase = pid * R

            # t-loads round-robin across issuing engines -> separate HWDGE
            # queues, so transfers overlap instead of serializing at depth 1
            dma_eng = [nc.sync, nc.scalar]

            # ---------- prefetch: first PF iterations' t-load + uconv ------
            uts = {}
            for it in range(PF):
                s, c = divmod(it, NCH)
                tt = tstream.tile([128, CH], F32, tag="t")
                for hh in range(2):
                    dma_eng[hh].dma_start(
                        out=tt[:, ts(hh, CH // 2)],
                        in_=tm[ts(s, 128), ds(c * CH + hh * CH // 2,
                                              CH // 2)])
                ut = upool.tile([128, CH], BF16, tag="u")
                nc.vector.tensor_scalar_sub(ut[:, :], tt[:, :], 0.5)
                uts[it] = ut

            # ---------- phase 1: v blocks + row thresholds ----------
            # per-group top-8 candidates pipeline with the psum copies; the
            # self column v_ii = |z_i|^2 is the strict row max (d2>0), so it
            # is masked to -BIG in the tiny candidate tile instead of vch
            # (A_ii=1 in phase 2 is corrected exactly on the host).
            for s in range(NSTRIP):
                v8g = work.tile([128, 8 * NPG], BF16, tag="v8g")
                for g in range(NPG):
                    ps = vps.tile([128, PG], F32, tag="vps")
                    for h in range(PG // CT):
                        c0 = g * PG + h * CT
                        nc.tensor.matmul(ps[:, ts(h, CT)], l2t[:, ts(s, 128)],
                                         ztb[:, ds(c0, CT)],
                                         start=True, stop=False)
                        nc.tensor.matmul(ps[:, ts(h, CT)], ones1[:, :],
                                         msq_row[:, ds(c0, CT)],
                                         start=False, stop=True)
                    nc.scalar.activation(vch[s][:, ts(g, PG)], ps[:, :],
                                         AF.Copy)
                    nc.vector.max(v8g[:, ts(g, 8)], vch[s][:, ts(g, PG)])

                pen = work.tile([128, 8 * NPG], BF16, tag="pen")
                nc.vector.tensor_scalar(pen[:, :], v8g[:, :],
                                        smargin[:, s:s + 1], mbig1[:, :],
                                        OP.is_ge, OP.mult)
                nc.vector.tensor_tensor(v8g[:, :], v8g[:, :], pen[:, :],
                                        OP.add)
                v8 = work.tile([128, 8], BF16, tag="v8")
                nc.vector.max(v8[:, :], v8g[:, :])
                nc.vector.tensor_copy(tauv[:, s:s + 1], v8[:, 7:8])
                mtd = work.tile([128, 1], F32, tag="mtd")
                nc.vector.tensor_tensor(mtd[:, :], tauv[:, s:s + 1],
                                        sqp[:, s:s + 1], OP.subtract)
                mtdb_s = work.tile([128, 1], BF16, tag="mtdb1")
                nc.vector.tensor_copy(mtdb_s[:, :], mtd[:, :])
                nc.sync.dma_start(out=cc_in[ts(s, 128)], in_=mtdb_s[:, :])

            # load-only prefetch: next 2 t-chunks issued before the
            # collective so the stream is not serialized behind it
            tts = {}
            for it in range(PF, PF + 2):
                s, c = divmod(it, NCH)
                tt = tstream.tile([128, CH], F32, tag="t")
                dma_eng[it % 2].dma_start(out=tt[:, :],
                                          in_=tm[ts(s, 128), ts(c, CH)])
                tts[it] = tt

            # ---------- all-gather thresholds (mtd_j = tauv_j - sq_j) ------
            if sim_nocc:
                for c in range(NCORES):
                    nc.sync.dma_start(out=cc_out[ts(c, R)], in_=cc_in[:])
            else:
                nc.gpsimd.collective_compute(
                    "AllGather", OP.bypass,
                    replica_groups=[list(range(NCORES))],
                    ins=[cc_in[:].opt()],
                    outs=[cc_out[:].opt()],
                )
            mtd_row = const.tile([1, N], BF16, tag="row8k")
            nc.sync.dma_start(out=mtd_row[:, :], in_=cc_out[:])

            mtdb = const.tile([128, N], BF16, tag="big8k")
            for g in range(4):
                nc.gpsimd.partition_broadcast(mtdb[:, ts(g, N // 4)],
                                              mtd_row[:, ts(g, N // 4)])

            # ---------- phase 2: masked accumulation ----------
            dpsum = dps.tile([128, 128], F32, name="dpsum")
            stps = dps.tile([1, 512], F32, name="stps")
            for it in range(NIT):
                s, c = divmod(it, NCH)
                if it in uts:
                    ut = uts[it]
                else:
                    if it in tts:
                        tt = tts[it]
                    else:
                        tt = tstream.tile([128, CH], F32, tag="t")
                        for hh in range(2):
                            dma_eng[hh].dma_start(
                                out=tt[:, ts(hh, CH // 2)],
                                in_=tm[ts(s, 128),
                                       ds(c * CH + hh * CH // 2, CH // 2)])
                    ut = upool.tile([128, CH], BF16, tag="u")
                    if it % 2 == 1:
                        nc.scalar.activation(ut[:, :], tt[:, :], AF.Copy,
                                             bias=-0.5)
                    else:
                        nc.vector.tensor_scalar_sub(ut[:, :], tt[:, :], 0.5)

                At = apool.tile([128, CH], BF16, tag="A")
                nc.vector.tensor_scalar(At[:, :], mtdb[:, ts(c, CH)],
                                        sqp[:, s:s + 1], tauv[:, s:s + 1],
                                        OP.add, OP.min)
                if debug_taps and it == 0:
                    nc.sync.dma_start(out=dbg["thr"][:, :], in_=At[:, :])
                    nc.sync.dma_start(out=dbg["u"][:, :], in_=ut[:, :])
                    nc.sync.dma_start(out=dbg["mtdb"][:, :],
                                      in_=mtdb[:, ts(c, CH)])
                    nc.sync.dma_start(out=dbg["v"][:, :],
                                      in_=vch[s][:, ts(c, CH)])
                    nc.sync.dma_start(out=dbg["tauv"][:, :], in_=tauv[:, :])
                nc.vector.tensor_tensor(At[:, :], vch[s][:, ts(c, CH)],
                                        At[:, :], OP.is_ge)
                if debug_taps and it == 0:
                    nc.sync.dma_start(out=dbg["A"][:, :], in_=At[:, :])

                for b in range(NB):
                    nc.tensor.matmul(dpsum[:, :], At[:, ts(b, 128)],
                                     ut[:, ts(b, 128)],
                                     start=(it == 0 and b == 0),
                                     stop=(it == NIT - 1 and b == NB - 1))
                for h in range(CH // 512):
                    nc.tensor.matmul(stps[:, :], ones_col[:, :],
                                     ut[:, ts(h, 512)],
                                     start=(it == 0 and h == 0),
                                     stop=(it == NIT - 1 and h == CH // 512 - 1))

            nc.scalar.activation(sau_sb[:, :], dpsum[:, :], AF.Copy)
            nc.scalar.activation(st_sb[:, :], stps[:, :], AF.Copy)
            nc.sync.dma_start(out=sau_out[:, :], in_=sau_sb[:, :])
            nc.sync.dma_start(out=st_out[:, :], in_=st_sb[:, :])

    nc.finalize()
    return nc


def _make_exec(nc):
    """Cached jitted SPMD executor (mirrors bass2jax.run_bass_via_pjrt)."""
    import jax
    from jax.sharding import Mesh, PartitionSpec
    try:
        from jax.experimental.shard_map import shard_map
    except Exception:
        from jax.sharding import shard_map  # newer jax
    from concourse import bass2jax

    bass2jax.install_neuronx_cc_hook()

    partition_name = (nc.partition_id_tensor.name
                      if nc.partition_id_tensor else None)
    in_names, out_names, out_avals, zero_out_shapes = [], [], [], []
    for alloc in nc.m.functions[0].allocations:
        if not isinstance(alloc, mybir.MemoryLocationSet):
            continue
        name = alloc.memorylocations[0].name
        if alloc.kind == "ExternalInput":
            if name != partition_name:
                in_names.append(name)
        elif alloc.kind == "ExternalOutput":
            shape = tuple(alloc.tensor_shape)
            dtype = mybir.dt.np(alloc.dtype)
            out_names.append(name)
            out_avals.append(jax.core.ShapedArray(shape, dtype))
            zero_out_shapes.append((shape, dtype))
    n_params = len(in_names)
    n_outs = len(out_names)
    all_in_names = list(in_names) + list(out_names)
    if partition_name is not None:
        all_in_names.append(partition_name)
    donate = tuple(range(n_params, n_params + n_outs))

    def _body(*args):
        operands = list(args)
        if partition_name is not None:
            operands.append(bass2jax.partition_id_tensor())
        outs = bass2jax._bass_exec_p.bind(
            *operands,
            out_avals=tuple(out_avals),
            in_names=tuple(all_in_names),
            out_names=tuple(out_names),
            lowering_input_output_aliases=(),
            sim_require_finite=True,
            sim_require_nnan=True,
            nc=nc,
        )
        return tuple(outs)

    devices = jax.devices()[:NCORES]
    mesh = Mesh(np.asarray(devices), ("core",))
    in_specs = (PartitionSpec("core"),) * (n_params + n_outs)
    out_specs = (PartitionSpec("core"),) * n_outs
    sharded = jax.jit(
        shard_map(_body, mesh=mesh, in_specs=in_specs, out_specs=out_specs,
                  check_rep=False),
        donate_argnums=donate, keep_unused=True)

    _CACHE["sharded"] = sharded
    _CACHE["in_names"] = in_names
    _CACHE["zero_out_shapes"] = zero_out_shapes
    _CACHE["out_names"] = out_names

    def runner(in_maps):
        concat_in = [np.concatenate([np.asarray(m[nm]) for m in in_maps],
                                    axis=0) for nm in in_names]
        zeros = [np.zeros((NCORES * sh[0],) + tuple(sh[1:]), dt)
                 for sh, dt in zero_out_shapes]
        out_arrs = sharded(*concat_in, *zeros)
        res = []
        for c in range(NCORES):
            d = {}
            for i, nm in enumerate(out_names):
                a = np.asarray(out_arrs[i])
                per = a.shape[0] // NCORES
                d[nm] = a[c * per:(c + 1) * per]
            res.append(d)
        return res

    return runner


def _get_runner():
    if "runner" not in _CACHE:
        nc = build()
        _CACHE["runner"] = _make_exec(nc)
    return _CACHE["runner"]


def _prep_inputs(Z, T):
    Z = np.ascontiguousarray(np.asarray(Z, dtype=np.float32))
    T = np.asarray(T)
    if T.dtype != np.float32:
        T = T.astype(np.float32)
    bf16 = ml_dtypes.bfloat16
    _CACHE["diag_corr"] = float(
        np.sum(np.diagonal(T).astype(np.float64) - 0.5))
    ZT = np.ascontiguousarray(Z.T)                       # [D, N] f32
    ztb = ZT.astype(bf16)                                # [128, N] bf16
    sq = np.sum(Z.astype(np.float64) * Z, axis=1).astype(np.float32)  # [N]
    msq = (-sq).astype(bf16)[None, :]                    # [1, N] bf16
    in_maps = []
    for c in range(NCORES):
        rows = slice(c * R, (c + 1) * R)
        l2t = np.ascontiguousarray((2.0 * ZT[:, rows]).astype(bf16))
        sqc = np.ascontiguousarray(
            sq[rows].reshape(NSTRIP, 128).T)             # [128, NSTRIP]
        in_maps.append({
            "ztb": ztb,
            "l2t": l2t,
            "msq": msq,
            "sq": sqc,
            "t": T[rows],
        })
    return in_maps


def assemble_loss(results):
    s_aw = 0.0
    s_w = 0.0
    for r in results:
        s_aw += float(np.asarray(r["sau"], dtype=np.float64)
                      .diagonal().sum())
        s_w += float(np.asarray(r["st"], dtype=np.float64).sum())
    # A_ii = 1 on device (self not masked in vch); remove its contribution
    s_aw -= _CACHE.get("diag_corr", 0.0)
    s_au = -2.0 * s_aw
    s_t = s_w + 0.5 * float(N) * N
    return np.float32(100.0 * (s_t + s_au) / (float(N) * N))


def kernel(Z, target_adj):
    runner = _get_runner()
    in_maps = _prep_inputs(Z, target_adj)
    results = runner(in_maps)
    return assemble_loss(results)


if __name__ == "__main__":
    rng = np.random.default_rng(0)
    Z = rng.standard_normal((N, D), dtype=np.float32)
    T = rng.random((N, N), dtype=np.float32)
    print("loss:", kernel(Z, T))
